# revision 13
# baseline (speedup 1.0000x reference)
"""LIF spiking-neuron recurrence kernel for Trainium2 (8 NeuronCores, SPMD).

Problem: x [32, 100, 8192] f32, decay [1] f32.
    d = sigmoid(decay)
    mem_0 = x[:,0];  mem_t = mem_{t-1} * d * (1 - spike_{t-1}) + x[:,t]
    spike_t = (mem_t > 0.5);  out[:,t] = spike_t  (f32 0/1)

Device formulation (bit-exact vs the reference):
    W_{-1} = 0
    M_t = (W_{t-1} * d) + x_t
    W_t = (M_t <= 0.5) * M_t
spike_t = (M_t > 0.5) = (W_t == 0) exactly. The recurrence runs as ONE
custom DVE op per chunk (out AP trails in0 by F elements in the same
persistent W buffer).

Output format: spikes leave the device BITPACKED 8 timesteps per byte
(0.41 MB/core instead of 3.3 MB/core; the kernel is HBM-bound on the
13.1 MB/core x load). Byte (p, g*256+f) bit k = spike at t=8g+k;
groups 0..11 are full, group 12 carries t=96..99 in bits 0..3.

Pipeline after the LIF chain, engine-balanced so nothing sits on DVE
besides the serial recurrence:
 1. Extraction: UNSCALED spike bytes s_t = (W_t == 0) as u8, written to
    a flat [128, T*256] buffer (one flat contiguous op per step-range).
    - ACT engine, groups 0..5: q = Square(1e19*W) into PSUM, then
      s = Relu(1 - q) -> u8 (exactly 1/0).
    - GPSIMD, groups 6..12: tensor_scalar is_equal (single-input Q7 op,
      ~1 cyc/elem; avoids DVE 2x_2P ops which would lock GPSIMD out of
      the shared SBUF port).
 2. Horner tree on uint16 bitcast views (2 packed bytes/slot, 2x_1p, no
    port contention), applying bit weights via scalar_tensor_tensor
    fused nodes out = in0*C + in1:
      L1 (GPSIMD): p01 = 2*s1 + s0, p23, p45, p67   per group
      L2 (DVE):    q0123 = 4*p23 + p01, q4567 = 4*p67 + p45
      L3 (DVE):    byte = 16*q4567 + q0123  -> packed acc
    All byte values stay < 256 and u16 slots < 65536, so the fp32
    internal ALU is exact.
Host unpacks with np.unpackbits.

Sharding: d-shard: core c owns d in [1024c, 1024c+1024); per-core
layout [128, T*256] with partition p = b*4 + (d_local//256). No
cross-core communication.
"""

from contextlib import ExitStack

import numpy as np

N_CORES = 8
B, T, D = 32, 100, 8192
P = 128          # SBUF partitions
F = 256          # free elements per timestep per core
THRESH = 0.5
NG_FULL = T // 8          # 12 full 8-step groups
NG = NG_FULL + 1          # 13 output groups
OUT_F = NG * F            # 3328 output bytes per partition

# Load chunks in timesteps; tail-light so little work trails the final
# (load-floor-gated) chunks.
SCHED = [8, 8, 16, 16, 16, 16, 8, 4, 4, 4]

_BUILD_CACHE: dict = {}
_LIF_OP = None


def _get_lif_op():
    """Register the fused LIF-step custom DVE op (idempotent)."""
    global _LIF_OP
    if _LIF_OP is not None:
        return _LIF_OP
    from concourse.dve_ops import (
        CUSTOM_DVE_SPECS, OPS, _SUB_OPCODE_FOR_NAME, DveOp,
    )
    from concourse.dve_spec import C0, C1, Spec, Src0, Src1, lower
    from concourse.dve_table_gen import dve_ver_for
    from concourse.dve_uop import DveOpSpec

    name = "LIF_STEP_ANT"
    if name in _SUB_OPCODE_FOR_NAME:
        _LIF_OP = next(op for op in OPS if op.name == name)
        return _LIF_OP

    M = Src0 * C0 + Src1

    def _ref(in0, in1, s0, s1, imm2):
        m = (in0.astype(np.float32) * np.float32(s0)
             + in1.astype(np.float32)).astype(np.float32)
        return np.where(m <= np.float32(s1), m, np.float32(0.0)).astype(np.float32)

    spec = Spec(body=M * (M <= C1), reference=_ref)
    row = max(_SUB_OPCODE_FOR_NAME.values()) + 1
    assert row < 0x20
    _SUB_OPCODE_FOR_NAME[name] = row
    shas = {}
    for ver in ("v3",):  # TRN2
        tmp = DveOpSpec(name=name, opcode=row, uops=lower(spec, ver=ver),
                        rd1_en=True)
        shas[ver] = tmp.sha(ver)
    assert dve_ver_for("TRN2") == "v3"
    op = DveOp(name, spec, subdim=False, uops_sha=shas)
    OPS.append(op)
    CUSTOM_DVE_SPECS[name] = spec
    _LIF_OP = op
    return op


def _build_nc(t_steps: int, d_imm: float):
    import concourse.tile as tile
    from concourse import bacc, bass, mybir

    assert t_steps == T, "schedule is hardcoded for T=100"
    lif_op = _get_lif_op()
    assert sum(SCHED) == T
    ends = []
    s = 0
    for tc in SCHED:
        s += tc
        ends.append(s)

    AF = mybir.ActivationFunctionType
    ALU = mybir.AluOpType

    nc = bacc.Bacc("TRN2", debug=False, target_bir_lowering=False)
    x_in = nc.dram_tensor("x", [P, T * F], mybir.dt.float32,
                          kind="ExternalInput")
    s_out = nc.dram_tensor("s", [P, OUT_F], mybir.dt.uint8,
                           kind="ExternalOutput")

    with tile.TileContext(nc) as tcx, ExitStack() as ctx:
        xpool = ctx.enter_context(tcx.tile_pool(name="xp", bufs=3))
        ppool = ctx.enter_context(tcx.tile_pool(name="pp", bufs=1))
        qpool = ctx.enter_context(
            tcx.tile_pool(name="qp", bufs=1, space=bass.MemorySpace.PSUM))
        spool = ctx.enter_context(tcx.tile_pool(name="sp", bufs=1))

        # Persistent state: W[:, t*F:(t+1)*F] holds W_{t-1} (slot 0 = 0).
        wbuf = spool.tile([P, (T + 1) * F], mybir.dt.float32)
        # Unscaled spike bytes, same step indexing as x.
        sbytes = spool.tile([P, T * F], mybir.dt.uint8)
        # Packed output accumulator.
        acc = spool.tile([P, OUT_F], mybir.dt.uint8)
        # ACT warmup scratch (pull the activation table load off the
        # critical path, overlapping the first DMA).
        scr = spool.tile([P, 32], mybir.dt.float32)

        nc.vector.memset(wbuf[:, 0:F], 0.0)
        nc.scalar.memzero(scr[:, :])
        nc.scalar.activation(out=scr[:, :], in_=scr[:, :], func=AF.Square)

        def u16(ap):
            return ap.bitcast(mybir.dt.uint16)

        def emit_act_extract(t_a, t_b):
            n = (t_b - t_a) * F
            qt = qpool.tile([P, 4096], mybir.dt.float32, tag="qt")
            nc.scalar.activation(out=qt[:, :n],
                                 in_=wbuf[:, (t_a + 1) * F:(t_b + 1) * F],
                                 func=AF.Square, scale=1e19)
            nc.scalar.activation(out=sbytes[:, t_a * F:t_b * F],
                                 in_=qt[:, :n], func=AF.Relu,
                                 bias=1.0, scale=-1.0)

        def emit_gp_extract(t_a, t_b):
            nc.gpsimd.tensor_scalar(
                out=sbytes[:, t_a * F:t_b * F],
                in0=wbuf[:, (t_a + 1) * F:(t_b + 1) * F],
                scalar1=0.0, scalar2=None, op0=ALU.is_equal)

        def emit_tree(bkey, g0, ng, nplanes=8):
            """DVE Horner tree, one stt op per level with the pair index
            as an AP dimension: byte = sum_k s_k 2^k, exact in u16/fp32."""
            sv = sbytes[:, 8 * g0 * F:(8 * g0 + 8 * (ng - 1) + nplanes) * F]
            pairs = ppool.tile([P, 6 * 4 * F], mybir.dt.uint8, tag="pairs",
                               name=f"pairs_{bkey}")
            if nplanes == 8:
                sv4 = sv.rearrange("p (g j two f) -> p g j two f",
                                   g=ng, j=4, two=2)
                pv = pairs[:, :ng * 4 * F].rearrange(
                    "p (g j f) -> p g j f", g=ng, j=4)
                nc.vector.scalar_tensor_tensor(
                    out=u16(pv), in0=u16(sv4[:, :, :, 1, :]), scalar=2.0,
                    in1=u16(sv4[:, :, :, 0, :]), op0=ALU.mult, op1=ALU.add)
                nib = ppool.tile([P, 6 * 2 * F], mybir.dt.uint8, tag="nib",
                                 name=f"nib_{bkey}")
                pv2 = pairs[:, :ng * 4 * F].rearrange(
                    "p (g h two f) -> p g h two f", g=ng, h=2, two=2)
                nv = nib[:, :ng * 2 * F].rearrange(
                    "p (g h f) -> p g h f", g=ng, h=2)
                nc.vector.scalar_tensor_tensor(
                    out=u16(nv), in0=u16(pv2[:, :, :, 1, :]), scalar=4.0,
                    in1=u16(pv2[:, :, :, 0, :]), op0=ALU.mult, op1=ALU.add)
                nv2 = nib[:, :ng * 2 * F].rearrange(
                    "p (g h f) -> p g h f", g=ng, h=2)
                av = acc[:, g0 * F:(g0 + ng) * F].rearrange(
                    "p (g f) -> p g f", g=ng)
                nc.vector.scalar_tensor_tensor(
                    out=u16(av), in0=u16(nv2[:, :, 1, :]), scalar=16.0,
                    in1=u16(nv2[:, :, 0, :]), op0=ALU.mult, op1=ALU.add)
            else:
                assert nplanes == 4 and ng == 1
                sv4 = sv.rearrange("p (g j two f) -> p g j two f",
                                   g=1, j=2, two=2)
                pv = pairs[:, :2 * F].rearrange("p (j f) -> p j f", j=2)
                nc.vector.scalar_tensor_tensor(
                    out=u16(pv), in0=u16(sv4[:, :, :, 1, :]).squeeze(1),
                    scalar=2.0, in1=u16(sv4[:, :, :, 0, :]).squeeze(1),
                    op0=ALU.mult, op1=ALU.add)
                nc.vector.scalar_tensor_tensor(
                    out=u16(acc[:, g0 * F:(g0 + 1) * F]),
                    in0=u16(pairs[:, F:2 * F]), scalar=4.0,
                    in1=u16(pairs[:, 0:F]), op0=ALU.mult, op1=ALU.add)

        # ---- emission schedule -----------------------------------------
        # chunk ends: [8,16,32,48,64,80,88,92,96,100]
        act_extract_at = {1: (0, 16), 2: (16, 32), 3: (32, 48)}
        gp_extract_at = {4: [(48, 64)], 5: [(64, 80)], 6: [(80, 88)],
                         8: [(88, 96)], 9: [(96, 100)]}
        trees_at = {6: [("B0", 0, 6, 8)], 8: [("B1", 6, 4, 8)],
                    9: [("B2", 10, 2, 8), ("REM", 12, 1, 4)]}

        t0 = 0
        for ci, tc in enumerate(SCHED):
            xt = xpool.tile([P, 16 * F], mybir.dt.float32, tag="xt")
            nc.sync.dma_start(out=xt[:, :tc * F],
                              in_=x_in[:, t0 * F:(t0 + tc) * F])
            nc.vector._custom_dve(
                lif_op,
                out=wbuf[:, (t0 + 1) * F:(t0 + tc + 1) * F],
                in0=wbuf[:, t0 * F:(t0 + tc) * F],
                in1=xt[:, :tc * F],
                s0=d_imm, s1=THRESH)
            if ci in act_extract_at:
                emit_act_extract(*act_extract_at[ci])
            for rng in gp_extract_at.get(ci, []):
                emit_gp_extract(*rng)
            for args in trees_at.get(ci, []):
                emit_tree(*args)
            t0 += tc

        # Stores: groups 0..5 unblock after B0; the rest at the end.
        nc.sync.dma_start(out=s_out[:, :6 * F], in_=acc[:, :6 * F])
        nc.sync.dma_start(out=s_out[:, 6 * F:], in_=acc[:, 6 * F:])
    nc.compile()
    return nc


def _get_nc(t_steps: int, d_imm: float):
    key = (t_steps, np.float32(d_imm).tobytes())
    if key not in _BUILD_CACHE:
        _BUILD_CACHE[key] = _build_nc(t_steps, d_imm)
    return _BUILD_CACHE[key]


def _shard_x(x: np.ndarray) -> list[np.ndarray]:
    b, t, d = x.shape
    # [b, t, core, chunk, 256] -> [core, b, chunk, t, 256] -> [core, 128, t*256]
    xr = x.reshape(b, t, N_CORES, 4, F).transpose(2, 0, 3, 1, 4)
    xr = np.ascontiguousarray(xr).reshape(N_CORES, P, t * F)
    return [xr[c] for c in range(N_CORES)]


def _unshard_spikes(s8: np.ndarray, t: int) -> np.ndarray:
    # s8: [core, 128, NG*256] packed bits; bit k of byte (p, g*256+f) is
    # spike at timestep 8g+k for lane (p, f).
    a = s8.reshape(N_CORES, P, NG, F, 1)
    bits = np.unpackbits(a, axis=-1, bitorder="little")  # [c, p, g, f, 8]
    bits = bits.transpose(0, 1, 2, 4, 3).reshape(N_CORES, P, NG * 8, F)
    bits = bits[:, :, :t, :]
    sr = bits.astype(np.float32).reshape(N_CORES, B, 4, t, F)
    sr = sr.transpose(1, 3, 0, 2, 4)
    return np.ascontiguousarray(sr).reshape(B, t, N_CORES * 4 * F)


def _sigmoid_f32(decay: np.ndarray) -> np.float32:
    import jax
    import jax.numpy as jnp
    d = np.asarray(jax.nn.sigmoid(jnp.asarray(decay, jnp.float32)))
    return np.float32(d.reshape(-1)[0])


def kernel(x: np.ndarray, decay: np.ndarray) -> np.ndarray:
    from concourse.bass_utils import run_bass_kernel_spmd

    x = np.asarray(x, dtype=np.float32)
    b, t, d = x.shape
    d_f32 = _sigmoid_f32(np.asarray(decay))

    nc = _get_nc(t, float(d_f32))
    shards = _shard_x(x)
    in_maps = [{"x": np.ascontiguousarray(s)} for s in shards]
    res = run_bass_kernel_spmd(nc, in_maps, core_ids=list(range(N_CORES)))
    s8 = np.stack([np.asarray(res.results[c]["s"]) for c in range(N_CORES)],
                  axis=0)
    return _unshard_spikes(s8, t)


# revision 14
# speedup vs baseline: 3.5094x; 3.5094x over previous
"""LIF spiking-neuron recurrence kernel for Trainium2 (8 NeuronCores, SPMD).

Problem: x [32, 100, 8192] f32, decay [1] f32.
    d = sigmoid(decay)
    mem_0 = x[:,0];  mem_t = mem_{t-1} * d * (1 - spike_{t-1}) + x[:,t]
    spike_t = (mem_t > 0.5);  out[:,t] = spike_t  (f32 0/1)

Device formulation (bit-exact vs the reference):
    W_{-1} = 0
    M_t = (W_{t-1} * d) + x_t
    W_t = (M_t <= 0.5) * M_t
spike_t = (M_t > 0.5) = (W_t == 0) exactly. The recurrence runs as ONE
custom DVE op per chunk (out AP trails in0 by F elements in the same
persistent W buffer).

Output format: spikes leave the device BITPACKED 8 timesteps per byte
(0.41 MB/core instead of 3.3 MB/core; the kernel is HBM-bound on the
13.1 MB/core x load). Byte (p, g*256+f) bit k = spike at t=8g+k;
groups 0..11 are full, group 12 carries t=96..99 in bits 0..3.

Pipeline after the LIF chain, engine-balanced so nothing sits on DVE
besides the serial recurrence:
 1. Extraction: UNSCALED spike bytes s_t = (W_t == 0) as u8, written to
    a flat [128, T*256] buffer (one flat contiguous op per step-range).
    - ACT engine, groups 0..5: q = Square(1e19*W) into PSUM, then
      s = Relu(1 - q) -> u8 (exactly 1/0).
    - GPSIMD, groups 6..12: tensor_scalar is_equal (single-input Q7 op,
      ~1 cyc/elem; avoids DVE 2x_2P ops which would lock GPSIMD out of
      the shared SBUF port).
 2. Horner tree on uint16 bitcast views (2 packed bytes/slot, 2x_1p, no
    port contention), applying bit weights via scalar_tensor_tensor
    fused nodes out = in0*C + in1:
      L1 (GPSIMD): p01 = 2*s1 + s0, p23, p45, p67   per group
      L2 (DVE):    q0123 = 4*p23 + p01, q4567 = 4*p67 + p45
      L3 (DVE):    byte = 16*q4567 + q0123  -> packed acc
    All byte values stay < 256 and u16 slots < 65536, so the fp32
    internal ALU is exact.
Host unpacks with np.unpackbits.

Sharding: d-shard: core c owns d in [1024c, 1024c+1024); per-core
layout [128, T*256] with partition p = b*4 + (d_local//256). No
cross-core communication.
"""

from contextlib import ExitStack

import numpy as np

N_CORES = 8
B, T, D = 32, 100, 8192
P = 128          # SBUF partitions
F = 256          # free elements per timestep per core
THRESH = 0.5
NG_FULL = T // 8          # 12 full 8-step groups
NG = NG_FULL + 1          # 13 output groups
OUT_F = NG * F            # 3328 output bytes per partition

# Load chunks in timesteps; tail-light so little work trails the final
# (load-floor-gated) chunks.
SCHED = [8, 8, 16, 16, 16, 16, 8, 4, 4, 4]

_BUILD_CACHE: dict = {}
_LIF_OP = None


def _get_lif_op():
    """Register the fused LIF-step custom DVE op (idempotent)."""
    global _LIF_OP
    if _LIF_OP is not None:
        return _LIF_OP
    from concourse.dve_ops import (
        CUSTOM_DVE_SPECS, OPS, _SUB_OPCODE_FOR_NAME, DveOp,
    )
    from concourse.dve_spec import C0, C1, Spec, Src0, Src1, lower
    from concourse.dve_table_gen import dve_ver_for
    from concourse.dve_uop import DveOpSpec

    name = "LIF_STEP_ANT"
    if name in _SUB_OPCODE_FOR_NAME:
        _LIF_OP = next(op for op in OPS if op.name == name)
        return _LIF_OP

    M = Src0 * C0 + Src1

    def _ref(in0, in1, s0, s1, imm2):
        m = (in0.astype(np.float32) * np.float32(s0)
             + in1.astype(np.float32)).astype(np.float32)
        return np.where(m <= np.float32(s1), m, np.float32(0.0)).astype(np.float32)

    spec = Spec(body=M * (M <= C1), reference=_ref)
    row = max(_SUB_OPCODE_FOR_NAME.values()) + 1
    assert row < 0x20
    _SUB_OPCODE_FOR_NAME[name] = row
    shas = {}
    for ver in ("v3",):  # TRN2
        tmp = DveOpSpec(name=name, opcode=row, uops=lower(spec, ver=ver),
                        rd1_en=True)
        shas[ver] = tmp.sha(ver)
    assert dve_ver_for("TRN2") == "v3"
    op = DveOp(name, spec, subdim=False, uops_sha=shas)
    OPS.append(op)
    CUSTOM_DVE_SPECS[name] = spec
    _LIF_OP = op
    return op


def _build_nc(t_steps: int, d_imm: float):
    import concourse.tile as tile
    from concourse import bacc, bass, mybir

    assert t_steps == T, "schedule is hardcoded for T=100"
    lif_op = _get_lif_op()
    assert sum(SCHED) == T
    ends = []
    s = 0
    for tc in SCHED:
        s += tc
        ends.append(s)

    AF = mybir.ActivationFunctionType
    ALU = mybir.AluOpType

    nc = bacc.Bacc("TRN2", debug=False, target_bir_lowering=False)
    x_in = nc.dram_tensor("x", [P, T * F], mybir.dt.float32,
                          kind="ExternalInput")
    s_out = nc.dram_tensor("s", [P, OUT_F], mybir.dt.uint8,
                           kind="ExternalOutput")

    with tile.TileContext(nc) as tcx, ExitStack() as ctx:
        xpool = ctx.enter_context(tcx.tile_pool(name="xp", bufs=3))
        ppool = ctx.enter_context(tcx.tile_pool(name="pp", bufs=1))
        qpool = ctx.enter_context(
            tcx.tile_pool(name="qp", bufs=1, space=bass.MemorySpace.PSUM))
        spool = ctx.enter_context(tcx.tile_pool(name="sp", bufs=1))

        # Persistent state: W[:, t*F:(t+1)*F] holds W_{t-1} (slot 0 = 0).
        wbuf = spool.tile([P, (T + 1) * F], mybir.dt.float32)
        # Unscaled spike bytes, same step indexing as x.
        sbytes = spool.tile([P, T * F], mybir.dt.uint8)
        # Packed output accumulator.
        acc = spool.tile([P, OUT_F], mybir.dt.uint8)
        # ACT warmup scratch (pull the activation table load off the
        # critical path, overlapping the first DMA).
        scr = spool.tile([P, 32], mybir.dt.float32)

        nc.vector.memset(wbuf[:, 0:F], 0.0)
        nc.scalar.memzero(scr[:, :])
        nc.scalar.activation(out=scr[:, :], in_=scr[:, :], func=AF.Square)

        def u16(ap):
            return ap.bitcast(mybir.dt.uint16)

        def emit_act_extract(t_a, t_b):
            n = (t_b - t_a) * F
            qt = qpool.tile([P, 4096], mybir.dt.float32, tag="qt")
            nc.scalar.activation(out=qt[:, :n],
                                 in_=wbuf[:, (t_a + 1) * F:(t_b + 1) * F],
                                 func=AF.Square, scale=1e19)
            nc.scalar.activation(out=sbytes[:, t_a * F:t_b * F],
                                 in_=qt[:, :n], func=AF.Relu,
                                 bias=1.0, scale=-1.0)

        def emit_dve_extract(t_a, t_b):
            nc.vector.tensor_scalar(
                out=sbytes[:, t_a * F:t_b * F],
                in0=wbuf[:, (t_a + 1) * F:(t_b + 1) * F],
                scalar1=0.0, scalar2=None, op0=ALU.is_equal)

        def emit_tree(bkey, g0, ng, nplanes=8):
            """DVE Horner tree, one stt op per level with the pair index
            as an AP dimension: byte = sum_k s_k 2^k, exact in u16/fp32."""
            sv = sbytes[:, 8 * g0 * F:(8 * g0 + 8 * (ng - 1) + nplanes) * F]
            pairs = ppool.tile([P, 6 * 4 * F], mybir.dt.uint8, tag="pairs",
                               name=f"pairs_{bkey}")
            if nplanes == 8:
                sv4 = sv.rearrange("p (g j two f) -> p g j two f",
                                   g=ng, j=4, two=2)
                pv = pairs[:, :ng * 4 * F].rearrange(
                    "p (g j f) -> p g j f", g=ng, j=4)
                nc.vector.scalar_tensor_tensor(
                    out=u16(pv), in0=u16(sv4[:, :, :, 1, :]), scalar=2.0,
                    in1=u16(sv4[:, :, :, 0, :]), op0=ALU.mult, op1=ALU.add)
                nib = ppool.tile([P, 6 * 2 * F], mybir.dt.uint8, tag="nib",
                                 name=f"nib_{bkey}")
                pv2 = pairs[:, :ng * 4 * F].rearrange(
                    "p (g h two f) -> p g h two f", g=ng, h=2, two=2)
                nv = nib[:, :ng * 2 * F].rearrange(
                    "p (g h f) -> p g h f", g=ng, h=2)
                nc.vector.scalar_tensor_tensor(
                    out=u16(nv), in0=u16(pv2[:, :, :, 1, :]), scalar=4.0,
                    in1=u16(pv2[:, :, :, 0, :]), op0=ALU.mult, op1=ALU.add)
                nv2 = nib[:, :ng * 2 * F].rearrange(
                    "p (g h f) -> p g h f", g=ng, h=2)
                av = acc[:, g0 * F:(g0 + ng) * F].rearrange(
                    "p (g f) -> p g f", g=ng)
                nc.vector.scalar_tensor_tensor(
                    out=u16(av), in0=u16(nv2[:, :, 1, :]), scalar=16.0,
                    in1=u16(nv2[:, :, 0, :]), op0=ALU.mult, op1=ALU.add)
            else:
                assert nplanes == 4 and ng == 1
                sv4 = sv.rearrange("p (g j two f) -> p g j two f",
                                   g=1, j=2, two=2)
                pv = pairs[:, :2 * F].rearrange("p (j f) -> p j f", j=2)
                nc.vector.scalar_tensor_tensor(
                    out=u16(pv), in0=u16(sv4[:, :, :, 1, :]).squeeze(1),
                    scalar=2.0, in1=u16(sv4[:, :, :, 0, :]).squeeze(1),
                    op0=ALU.mult, op1=ALU.add)
                nc.vector.scalar_tensor_tensor(
                    out=u16(acc[:, g0 * F:(g0 + 1) * F]),
                    in0=u16(pairs[:, F:2 * F]), scalar=4.0,
                    in1=u16(pairs[:, 0:F]), op0=ALU.mult, op1=ALU.add)

        # ---- emission schedule -----------------------------------------
        # chunk ends: [8,16,32,48,64,80,88,92,96,100]
        # ACT covers g0-7; DVE is_equal (2x_2p) covers g8-12 (no GPSIMD:
        # its tensor ops run ~18 cyc/elem and lock the shared SBUF port,
        # which stalls DVE).
        act_extract_at = {1: (0, 16), 2: (16, 32), 3: (32, 48),
                          4: (48, 64)}
        dve_extract_at = {6: [(64, 80)], 7: [(80, 88)],
                          9: [(88, 96), (96, 100)]}
        trees_at = {5: [("B0", 0, 6, 8)], 7: [("B2", 8, 2, 8)],
                    9: [("B1", 6, 2, 8), ("B3", 10, 2, 8),
                        ("REM", 12, 1, 4)]}

        t0 = 0
        for ci, tc in enumerate(SCHED):
            xt = xpool.tile([P, 16 * F], mybir.dt.float32, tag="xt")
            nc.sync.dma_start(out=xt[:, :tc * F],
                              in_=x_in[:, t0 * F:(t0 + tc) * F])
            nc.vector._custom_dve(
                lif_op,
                out=wbuf[:, (t0 + 1) * F:(t0 + tc + 1) * F],
                in0=wbuf[:, t0 * F:(t0 + tc) * F],
                in1=xt[:, :tc * F],
                s0=d_imm, s1=THRESH)
            if ci in act_extract_at:
                emit_act_extract(*act_extract_at[ci])
            for rng in dve_extract_at.get(ci, []):
                emit_dve_extract(*rng)
            for args in trees_at.get(ci, []):
                emit_tree(*args)
            t0 += tc

        # Stores: groups 0..5 unblock after B0; the rest at the end.
        nc.sync.dma_start(out=s_out[:, :6 * F], in_=acc[:, :6 * F])
        nc.sync.dma_start(out=s_out[:, 6 * F:], in_=acc[:, 6 * F:])
    nc.compile()
    return nc


def _get_nc(t_steps: int, d_imm: float):
    key = (t_steps, np.float32(d_imm).tobytes())
    if key not in _BUILD_CACHE:
        _BUILD_CACHE[key] = _build_nc(t_steps, d_imm)
    return _BUILD_CACHE[key]


def _shard_x(x: np.ndarray) -> list[np.ndarray]:
    b, t, d = x.shape
    # [b, t, core, chunk, 256] -> [core, b, chunk, t, 256] -> [core, 128, t*256]
    xr = x.reshape(b, t, N_CORES, 4, F).transpose(2, 0, 3, 1, 4)
    xr = np.ascontiguousarray(xr).reshape(N_CORES, P, t * F)
    return [xr[c] for c in range(N_CORES)]


def _unshard_spikes(s8: np.ndarray, t: int) -> np.ndarray:
    # s8: [core, 128, NG*256] packed bits; bit k of byte (p, g*256+f) is
    # spike at timestep 8g+k for lane (p, f).
    a = s8.reshape(N_CORES, P, NG, F, 1)
    bits = np.unpackbits(a, axis=-1, bitorder="little")  # [c, p, g, f, 8]
    bits = bits.transpose(0, 1, 2, 4, 3).reshape(N_CORES, P, NG * 8, F)
    bits = bits[:, :, :t, :]
    sr = bits.astype(np.float32).reshape(N_CORES, B, 4, t, F)
    sr = sr.transpose(1, 3, 0, 2, 4)
    return np.ascontiguousarray(sr).reshape(B, t, N_CORES * 4 * F)


def _sigmoid_f32(decay: np.ndarray) -> np.float32:
    import jax
    import jax.numpy as jnp
    d = np.asarray(jax.nn.sigmoid(jnp.asarray(decay, jnp.float32)))
    return np.float32(d.reshape(-1)[0])


def kernel(x: np.ndarray, decay: np.ndarray) -> np.ndarray:
    from concourse.bass_utils import run_bass_kernel_spmd

    x = np.asarray(x, dtype=np.float32)
    b, t, d = x.shape
    d_f32 = _sigmoid_f32(np.asarray(decay))

    nc = _get_nc(t, float(d_f32))
    shards = _shard_x(x)
    in_maps = [{"x": np.ascontiguousarray(s)} for s in shards]
    res = run_bass_kernel_spmd(nc, in_maps, core_ids=list(range(N_CORES)))
    s8 = np.stack([np.asarray(res.results[c]["s"]) for c in range(N_CORES)],
                  axis=0)
    return _unshard_spikes(s8, t)


# revision 15
# speedup vs baseline: 4.4814x; 1.2770x over previous
"""LIF spiking-neuron recurrence kernel for Trainium2 (8 NeuronCores, SPMD).

Problem: x [32, 100, 8192] f32, decay [1] f32.
    d = sigmoid(decay)
    mem_0 = x[:,0];  mem_t = mem_{t-1} * d * (1 - spike_{t-1}) + x[:,t]
    spike_t = (mem_t > 0.5);  out[:,t] = spike_t  (f32 0/1)

Device formulation (bit-exact vs the reference):
    W_{-1} = 0
    M_t = (W_{t-1} * d) + x_t
    W_t = (M_t <= 0.5) * M_t
spike_t = (M_t > 0.5) = (W_t == 0) exactly. The recurrence runs as ONE
custom DVE op per chunk over a persistent W buffer (out AP trails in0 by
F elements; the written state is read back ~250 cycles later).

Spike extraction/output — engine-balanced around the measured limits
(DVE is the scarce engine: the serial LIF chain alone costs ~27 us; the
HBM load floor is ~37 us; ACT runs 1 elem/cyc/lane with 2 passes needed
for a compare; GPSIMD tensor ops are ~18 cyc/elem and lock the shared
SBUF port, so it is not used):
  - ACT chunk-shares (early steps of each chunk): q = Square(1e19*W)
    into PSUM, s = Relu(1 - q) -> u8 {0,1}; stored as plain bytes.
  - DVE share (rest): custom SPIKE_PAIR op, one 1x pass fusing
    extraction and 2-bit packing: byte = (W_even==0) + 2*(W_odd==0),
    storing HALF the bytes for the same DVE cost as a plain is_equal.
Two output tensors (u8 spikes / 2-bit pairs); host reassembles.

Loads go on the sync HWDGE ring (nothing else queues there), stores on
the scalar ring interleaved with ACT compute; DVE has NO cross-engine
input dependencies (pair ops read W which DVE itself wrote), so the
serial LIF chain is never blocked.

Sharding: d-shard: core c owns d in [1024c, 1024c+1024); per-core
layout [128, T*256] with partition p = b*4 + (d_local//256). No
cross-core communication.
"""

from contextlib import ExitStack

import numpy as np

N_CORES = 8
B, T, D = 32, 100, 8192
P = 128          # SBUF partitions
F = 256          # free elements per timestep per core
THRESH = 0.5

# Load chunks (timesteps) and the per-chunk ACT share (first act_c steps
# extracted by ACT as u8; the rest go through the DVE pair op, so the
# non-ACT count must be even). Tail chunks are small and all-DVE so the
# post-load-floor serial tail stays short.
SCHED = [4, 12, 20, 20, 20, 16, 4, 4]
ACT_STEPS = [4, 8, 8, 8, 8, 8, 0, 0]

_BUILD_CACHE: dict = {}
_LIF_OP = None
_PAIR_OP = None


def _get_custom_ops():
    """Register the fused LIF-step and spike-pair DVE ops (idempotent)."""
    global _LIF_OP, _PAIR_OP
    if _LIF_OP is not None:
        return _LIF_OP, _PAIR_OP
    from concourse.dve_ops import (
        CUSTOM_DVE_SPECS, OPS, _SUB_OPCODE_FOR_NAME, DveOp,
    )
    from concourse.dve_spec import C0, C1, Spec, Src0, Src1, eq, lower
    from concourse.dve_table_gen import dve_ver_for
    from concourse.dve_uop import DveOpSpec

    assert dve_ver_for("TRN2") == "v3"

    def register(name, spec):
        if name in _SUB_OPCODE_FOR_NAME:
            return next(op for op in OPS if op.name == name)
        row = max(_SUB_OPCODE_FOR_NAME.values()) + 1
        assert row < 0x20
        _SUB_OPCODE_FOR_NAME[name] = row
        tmp = DveOpSpec(name=name, opcode=row, uops=lower(spec, ver="v3"),
                        rd1_en=True)
        op = DveOp(name, spec, subdim=False, uops_sha={"v3": tmp.sha("v3")})
        OPS.append(op)
        CUSTOM_DVE_SPECS[name] = spec
        return op

    M = Src0 * C0 + Src1

    def _lif_ref(in0, in1, s0, s1, imm2):
        m = (in0.astype(np.float32) * np.float32(s0)
             + in1.astype(np.float32)).astype(np.float32)
        return np.where(m <= np.float32(s1), m,
                        np.float32(0.0)).astype(np.float32)

    _LIF_OP = register("LIF_STEP_ANT",
                       Spec(body=M * (M <= C1), reference=_lif_ref))

    def _pair_ref(in0, in1, s0, s1, imm2):
        return ((in0 == np.float32(s0)).astype(np.float32)
                + (in1 == np.float32(s0)).astype(np.float32)
                * np.float32(s1)).astype(np.float32)

    _PAIR_OP = register("SPIKE_PAIR_ANT",
                        Spec(body=eq(Src0, C0) + eq(Src1, C0) * C1,
                             reference=_pair_ref))
    return _LIF_OP, _PAIR_OP


def _splits():
    """Per-chunk (t0, tc, act_c, pair_c, u8_off, p2_off) in elements."""
    out = []
    t0 = u8o = p2o = 0
    for tc, ac in zip(SCHED, ACT_STEPS):
        pc = tc - ac
        assert pc % 2 == 0
        out.append((t0, tc, ac, pc, u8o, p2o))
        t0 += tc
        u8o += ac * F
        p2o += (pc // 2) * F
    return out, u8o, p2o


def _build_nc(t_steps: int, d_imm: float):
    import concourse.tile as tile
    from concourse import bacc, bass, mybir

    assert t_steps == T, "schedule is hardcoded for T=100"
    lif_op, pair_op = _get_custom_ops()
    assert sum(SCHED) == T
    chunks, u8_total, p2_total = _splits()

    AF = mybir.ActivationFunctionType

    nc = bacc.Bacc("TRN2", debug=False, target_bir_lowering=False)
    x_in = nc.dram_tensor("x", [P, T * F], mybir.dt.float32,
                          kind="ExternalInput")
    su8_out = nc.dram_tensor("su8", [P, u8_total], mybir.dt.uint8,
                             kind="ExternalOutput")
    sp2_out = nc.dram_tensor("sp2", [P, p2_total], mybir.dt.uint8,
                             kind="ExternalOutput")

    max_tc = max(SCHED)
    max_ac = max(ACT_STEPS)
    max_pc = max(tc - ac for tc, ac in zip(SCHED, ACT_STEPS))

    with tile.TileContext(nc) as tcx, ExitStack() as ctx:
        xpool = ctx.enter_context(tcx.tile_pool(name="xp", bufs=3))
        opool = ctx.enter_context(tcx.tile_pool(name="op", bufs=2))
        qpool = ctx.enter_context(
            tcx.tile_pool(name="qp", bufs=1, space=bass.MemorySpace.PSUM))
        spool = ctx.enter_context(tcx.tile_pool(name="sp", bufs=1))

        # Persistent state: W[:, t*F:(t+1)*F] holds W_{t-1} (slot 0 = 0).
        wbuf = spool.tile([P, (T + 1) * F], mybir.dt.float32)
        scr = spool.tile([P, 32], mybir.dt.float32)

        nc.vector.memset(wbuf[:, 0:F], 0.0)
        nc.scalar.memzero(scr[:, :])
        nc.scalar.activation(out=scr[:, :], in_=scr[:, :], func=AF.Square)

        def emit_act_extract(t0, ac, u8o):
            n = ac * F
            qt = qpool.tile([P, max_ac * F], mybir.dt.float32, tag="qt")
            st = opool.tile([P, max_ac * F], mybir.dt.uint8, tag="su")
            nc.scalar.activation(out=qt[:, :n],
                                 in_=wbuf[:, (t0 + 1) * F:(t0 + ac + 1) * F],
                                 func=AF.Square, scale=1e19)
            nc.scalar.activation(out=st[:, :n], in_=qt[:, :n], func=AF.Relu,
                                 bias=1.0, scale=-1.0)
            nc.scalar.dma_start(out=su8_out[:, u8o:u8o + n], in_=st[:, :n])

        def emit_pair(t0, ac, pc, p2o):
            n = (pc // 2) * F
            pt = opool.tile([P, (max_pc // 2) * F], mybir.dt.uint8, tag="sp")
            wv = wbuf[:, (t0 + ac + 1) * F:(t0 + ac + pc + 1) * F]
            wv = wv.rearrange("p (g two f) -> p g two f", g=pc // 2, two=2)
            ov = pt[:, :n].rearrange("p (g f) -> p g f", g=pc // 2)
            nc.vector._custom_dve(pair_op, out=ov, in0=wv[:, :, 0, :],
                                  in1=wv[:, :, 1, :], s0=0.0, s1=2.0)
            nc.scalar.dma_start(out=sp2_out[:, p2o:p2o + n], in_=pt[:, :n])

        prev = None
        for ci, (t0, tc, ac, pc, u8o, p2o) in enumerate(chunks):
            xt = xpool.tile([P, max_tc * F], mybir.dt.float32, tag="xt")
            nc.sync.dma_start(out=xt[:, :tc * F],
                              in_=x_in[:, t0 * F:(t0 + tc) * F])
            nc.vector._custom_dve(
                lif_op,
                out=wbuf[:, (t0 + 1) * F:(t0 + tc + 1) * F],
                in0=wbuf[:, t0 * F:(t0 + tc) * F],
                in1=xt[:, :tc * F],
                s0=d_imm, s1=THRESH)
            if prev is not None:
                pt0, _, pac, ppc, pu8o, pp2o = prev
                if pac:
                    emit_act_extract(pt0, pac, pu8o)
                if ppc:
                    emit_pair(pt0, pac, ppc, pp2o)
            prev = chunks[ci]
        pt0, _, pac, ppc, pu8o, pp2o = prev
        if pac:
            emit_act_extract(pt0, pac, pu8o)
        if ppc:
            emit_pair(pt0, pac, ppc, pp2o)
    nc.compile()
    return nc


def _get_nc(t_steps: int, d_imm: float):
    key = (t_steps, np.float32(d_imm).tobytes())
    if key not in _BUILD_CACHE:
        _BUILD_CACHE[key] = _build_nc(t_steps, d_imm)
    return _BUILD_CACHE[key]


def _shard_x(x: np.ndarray) -> list[np.ndarray]:
    b, t, d = x.shape
    # [b, t, core, chunk, 256] -> [core, b, chunk, t, 256] -> [core, 128, t*256]
    xr = x.reshape(b, t, N_CORES, 4, F).transpose(2, 0, 3, 1, 4)
    xr = np.ascontiguousarray(xr).reshape(N_CORES, P, t * F)
    return [xr[c] for c in range(N_CORES)]


def _unshard_spikes(su8: np.ndarray, sp2: np.ndarray, t: int) -> np.ndarray:
    # Reassemble per-step spike bytes [core, 128, T, F] from the two
    # output formats, then unshard to [B, T, D].
    chunks, _, _ = _splits()
    sp = np.empty((N_CORES, P, t, F), dtype=np.uint8)
    for t0, tc, ac, pc, u8o, p2o in chunks:
        if ac:
            sp[:, :, t0:t0 + ac, :] = su8[:, :, u8o:u8o + ac * F].reshape(
                N_CORES, P, ac, F)
        if pc:
            pb = sp2[:, :, p2o:p2o + (pc // 2) * F].reshape(
                N_CORES, P, pc // 2, F)
            sp[:, :, t0 + ac:t0 + tc:2, :] = pb & 1
            sp[:, :, t0 + ac + 1:t0 + tc:2, :] = pb >> 1
    sr = sp.astype(np.float32).reshape(N_CORES, B, 4, t, F)
    sr = sr.transpose(1, 3, 0, 2, 4)
    return np.ascontiguousarray(sr).reshape(B, t, N_CORES * 4 * F)


def _sigmoid_f32(decay: np.ndarray) -> np.float32:
    import jax
    import jax.numpy as jnp
    d = np.asarray(jax.nn.sigmoid(jnp.asarray(decay, jnp.float32)))
    return np.float32(d.reshape(-1)[0])


def kernel(x: np.ndarray, decay: np.ndarray) -> np.ndarray:
    from concourse.bass_utils import run_bass_kernel_spmd

    x = np.asarray(x, dtype=np.float32)
    b, t, d = x.shape
    d_f32 = _sigmoid_f32(np.asarray(decay))

    nc = _get_nc(t, float(d_f32))
    shards = _shard_x(x)
    in_maps = [{"x": np.ascontiguousarray(s)} for s in shards]
    res = run_bass_kernel_spmd(nc, in_maps, core_ids=list(range(N_CORES)))
    su8 = np.stack([np.asarray(res.results[c]["su8"])
                    for c in range(N_CORES)], axis=0)
    sp2 = np.stack([np.asarray(res.results[c]["sp2"])
                    for c in range(N_CORES)], axis=0)
    return _unshard_spikes(su8, sp2, t)
